# revision 1
# baseline (speedup 1.0000x reference)
"""VQ codebook nearest-neighbor kernel for Trainium2 (8 NeuronCores).

Problem: embeddings (16, 4096, 64) f32, codebook (1024, 64) f32.
Output: argmin_j ||e - c_j||^2 -> (16, 4096) int32.

Math: argmin_j (||c_j||^2 - 2 e.c_j) == argmax_j (2 e.c_j - ||c_j||^2).

Sharding: data-parallel over flattened N = B*S, 8192 rows per core;
codebook replicated.

Per-core kernel (rows on partitions, codes on free dim):
  - 2-block row-group packing: row-tiles t and t+32 run CONCURRENTLY on
    PE row-groups 0-1 (SBUF partitions 0-63) and 2-3 (partitions
    64-127). Measured on HW: paired matmuls issue ~4ns apart and stream
    together (2x cold, and the pair rate reaches ~259ns warm).
  - Exact fp32-grade scores via bf16 hi/lo split accumulated in fp32
    PSUM: e.c = e_hi.c_hi + e_hi.c_lo + e_lo.c_hi + e_lo.c_lo (products
    of bf16 pairs are exact in fp32). The -||c||^2 bias rides a 5th
    K=3 stream: ones(3) x three successive bf16 residuals of -||c||^2.
  - ScalarE evacuates PSUM -> SBUF fp32.
  - VectorE max (top-8) + max_index -> per-row argmax index (uint32).
  - Results staged in SBUF, compacted with one DVE copy, written back
    with one contiguous DMA (a strided 4B-element DMA here costs ~20us).

Measured on HW (neuron-profile, max over 8 cores): ~171us (fast chip
state; up to ~207us when the chip's power state throttles engine
clocks ~25%), 1/65536 index mismatch vs the fp32 reference -- the one
row whose top-2 distance gap is 3.9e-6, below fp32 rounding noise for
any evaluation order.

Engine occupancy at ~171us: VectorE ~158us busy (max+max_index at
1 elem/cycle/partition is the ISA floor; tensor_tensor_reduce crashes
this HW path, tensor_tensor_scan measures 2.9ns/step, fp16/bf16
scanning breaks exactness), TensorE ~109us (640 paired matmuls, pairs
stream concurrently on disjoint row-groups), ScalarE ~63us (one
evacuation per pair), ~12us DMA/table-load head; the drain/sem-reset
epilogue mostly overlaps the tail.
"""

import os
import sys

for _p in ("/opt/trn_rl_repo", "/root/.axon_site/_ro/trn_rl_repo"):
    if os.path.isdir(_p) and _p not in sys.path:
        sys.path.append(_p)

import numpy as np

import concourse.bacc as bacc
import concourse.bass as bass
import concourse.mybir as mybir
from concourse.bass_utils import run_bass_kernel_spmd
from concourse.tile import TileContext

B, S, D = 16, 4096, 64
A = 1024                     # num codes
N_CORES = 8
N_TOTAL = B * S              # 65536
N_PER_CORE = N_TOTAL // N_CORES   # 8192
ROW_TILE = 128
F32 = mybir.dt.float32
U32 = mybir.dt.uint32
BF16 = mybir.dt.bfloat16
N_SPLITS = 4                 # hi.hi, hi.lo, lo.hi, lo.lo


def build_nc(n_rows: int = N_PER_CORE, dma_chunks: int = 8) -> bass.Bass:
    """Build the per-core Bass module (same program on all 8 cores)."""
    n_tiles = n_rows // ROW_TILE          # 64
    n_pairs = n_tiles // 2                # 32
    half_rows = n_rows // 2               # 4096
    nc = bacc.Bacc()
    # 2-block packed: partitions 0-63 = rows [0, n/2), 64-127 = [n/2, n)
    et_hi = nc.declare_dram_parameter("et_hi", [128, half_rows], BF16,
                                      isOutput=False)
    et_lo = nc.declare_dram_parameter("et_lo", [128, half_rows], BF16,
                                      isOutput=False)
    # [:, 0:A] = c_hi (dup at partitions 0-63 / 64-127), [:, A:2A] = c_lo
    cbt = nc.declare_dram_parameter("cbt", [128, 2 * A], BF16,
                                    isOutput=False)
    # rows 0-2 and 64-66 = three bf16 residuals of -||c||^2, rest zero
    bq = nc.declare_dram_parameter("bq", [128, A], BF16, isOutput=False)
    idx = nc.declare_dram_parameter("idx", [n_rows], U32, isOutput=True)

    with TileContext(nc) as tc:
        with (
            tc.tile_pool(name="const", bufs=1) as const_pool,
            tc.tile_pool(name="etp", bufs=2 * dma_chunks) as et_pool,
            tc.tile_pool(name="ps", bufs=2, space="PSUM") as psum_pool,
            tc.tile_pool(name="sc", bufs=3) as sc_pool,
            tc.tile_pool(name="m8", bufs=4) as m8_pool,
        ):
            cb = const_pool.tile([128, 2 * A], BF16)
            nc.sync.dma_start(out=cb, in_=cbt[:, :])
            bqt = const_pool.tile([128, A], BF16)
            nc.sync.dma_start(out=bqt, in_=bq[:, :])
            ones = const_pool.tile([128, ROW_TILE], BF16)
            nc.vector.memset(ones[:, :], 1.0)
            stage = const_pool.tile([ROW_TILE, n_tiles * 8], U32)

            cols_per_chunk = half_rows // dma_chunks       # 1024
            pairs_per_chunk = cols_per_chunk // ROW_TILE   # 8
            e_tiles = []
            for ci in range(dma_chunks):
                sl = slice(ci * cols_per_chunk, (ci + 1) * cols_per_chunk)
                thi = et_pool.tile([128, cols_per_chunk], BF16, tag="ehi")
                nc.sync.dma_start(out=thi, in_=et_hi[:, sl])
                tlo = et_pool.tile([128, cols_per_chunk], BF16, tag="elo")
                nc.sync.dma_start(out=tlo, in_=et_lo[:, sl])
                e_tiles.append((thi, tlo))

            for pt in range(n_pairs):
                ci, local = divmod(pt, pairs_per_chunk)
                csl = slice(local * ROW_TILE, (local + 1) * ROW_TILE)
                ehi, elo = e_tiles[ci]
                ps = psum_pool.tile([ROW_TILE, 2 * A], F32)
                for h in range(2):
                    hsA = slice(h * 512, (h + 1) * 512)
                    hsB = slice(A + h * 512, A + (h + 1) * 512)
                    # stream 0: bias (K=3 ones x -||c||^2 residuals)
                    nc.tensor.matmul(ps[:, hsA], ones[0:3, :],
                                     bqt[0:3, h * 512:(h + 1) * 512],
                                     start=True, stop=False)
                    nc.tensor.matmul(ps[:, hsB], ones[64:67, :],
                                     bqt[64:67, h * 512:(h + 1) * 512],
                                     start=True, stop=False)
                    # streams 1-4: bf16 split products
                    combos = (
                        (ehi, 0), (ehi, A), (elo, 0), (elo, A),
                    )[:N_SPLITS]
                    for si, (e_t, coff) in enumerate(combos):
                        last = si == len(combos) - 1
                        co = slice(coff + h * 512, coff + h * 512 + 512)
                        nc.tensor.matmul(
                            ps[:, hsA], e_t[0:64, csl], cb[0:64, co],
                            start=False, stop=last)
                        nc.tensor.matmul(
                            ps[:, hsB], e_t[64:128, csl], cb[64:128, co],
                            start=False, stop=last)
                # one evacuation for both tiles of the pair (pair 0 split
                # in two so the first max starts one evac earlier)
                sc = sc_pool.tile([ROW_TILE, 2 * A], F32)
                if pt == 0:
                    nc.scalar.copy(out=sc[:, 0:A], in_=ps[:, 0:A])
                    nc.scalar.copy(out=sc[:, A:], in_=ps[:, A:])
                else:
                    nc.scalar.copy(out=sc[:, :], in_=ps[:, :])
                for ti, base in ((pt, 0), (pt + n_pairs, A)):
                    m8 = m8_pool.tile([ROW_TILE, 8], F32)
                    nc.vector.max(out=m8[:, :], in_=sc[:, base:base + A])
                    nc.vector.max_index(
                        out=stage[:, ti * 8:(ti + 1) * 8],
                        in_max=m8[:, :],
                        in_values=sc[:, base:base + A],
                    )

            # compact the strided stage on DVE (one ~130ns op), then one
            # contiguous DMA; host undoes the (p, t) layout.
            compact = m8_pool.tile([ROW_TILE, n_tiles], U32, tag="compact")
            nc.vector.tensor_copy(
                out=compact[:, :],
                in_=stage.rearrange("p (t e) -> p t e", e=8)[:, :, 0])
            idx_view = idx.rearrange("(p t) -> p t", t=n_tiles)
            nc.sync.dma_start(out=idx_view, in_=compact[:, :])
    nc.compile()
    return nc


def _bf16_split(x64: np.ndarray, n: int):
    """Successive bf16 residuals: sum(parts) ~= x to ~2^-(9n) relative."""
    import ml_dtypes
    parts = []
    resid = x64.astype(np.float64)
    for _ in range(n):
        p = resid.astype(np.float32).astype(ml_dtypes.bfloat16)
        parts.append(p)
        resid = resid - p.astype(np.float64)
    return parts


def make_in_maps(embeddings: np.ndarray, codebook: np.ndarray,
                 n_rows: int = N_PER_CORE, n_cores: int = N_CORES):
    """Host-side sharding/layout prep (2-block packed)."""
    import ml_dtypes
    flat = np.asarray(embeddings, dtype=np.float32).reshape(-1, D)
    cb = np.asarray(codebook, dtype=np.float32)

    two_ct = 2.0 * cb.T.astype(np.float64)                    # (D, A)
    ct_hi, ct_lo = _bf16_split(two_ct, 2)
    cbt = np.zeros((128, 2 * A), dtype=ml_dtypes.bfloat16)
    cbt[0:D, 0:A] = ct_hi
    cbt[64:64 + D, 0:A] = ct_hi
    cbt[0:D, A:2 * A] = ct_lo
    cbt[64:64 + D, A:2 * A] = ct_lo

    cbsq = (cb.astype(np.float64) ** 2).sum(axis=1)           # (A,)
    q_parts = _bf16_split(-cbsq, 3)
    bq = np.zeros((128, A), dtype=ml_dtypes.bfloat16)
    for i, qp in enumerate(q_parts):
        bq[i] = qp
        bq[64 + i] = qp

    e64 = flat.T.astype(np.float64)                           # (D, N)
    e_hi, e_lo = _bf16_split(e64, 2)

    half = n_rows // 2
    in_maps = []
    for c in range(n_cores):
        r0 = c * n_rows
        eh = np.zeros((128, half), dtype=ml_dtypes.bfloat16)
        el = np.zeros((128, half), dtype=ml_dtypes.bfloat16)
        eh[0:D] = e_hi[:, r0:r0 + half]
        eh[64:64 + D] = e_hi[:, r0 + half:r0 + n_rows]
        el[0:D] = e_lo[:, r0:r0 + half]
        el[64:64 + D] = e_lo[:, r0 + half:r0 + n_rows]
        in_maps.append({
            "et_hi": np.ascontiguousarray(eh),
            "et_lo": np.ascontiguousarray(el),
            "cbt": cbt,
            "bq": bq,
        })
    return in_maps


_NC_CACHE: dict = {}


def _get_nc():
    key = N_PER_CORE
    if key not in _NC_CACHE:
        _NC_CACHE[key] = build_nc()
    return _NC_CACHE[key]


def kernel(embeddings: np.ndarray, codebook: np.ndarray, *,
           trace: bool = False, **run_kwargs) -> np.ndarray:
    nc = _get_nc()
    in_maps = make_in_maps(embeddings, codebook)
    res = run_bass_kernel_spmd(nc, in_maps, core_ids=list(range(N_CORES)),
                               trace=trace, **run_kwargs)
    n_tiles = N_PER_CORE // ROW_TILE
    out = np.concatenate(
        [res.results[c]["idx"].reshape(ROW_TILE, n_tiles).T.reshape(-1)
         for c in range(N_CORES)])
    out = out.astype(np.int32).reshape(B, S)
    if trace:
        kernel.last_results = res
    return out

